# revision 48
# baseline (speedup 1.0000x reference)
"""Masked multi-head attention (fused QKV) on 8 trn2 NeuronCores.

Problem (full shapes): x [2, 2048, 1024] f32, W [3072, 1024], b [3072].
  z = x @ W.T + b ; k,q,v = split(z) ; heads H=16, hd=64
  out = softmax(causal(q k^T / sqrt(1024))) v   -> [2, 2048, 1024]

Sharding: core c handles batch n=c//4 and head group g=c%4 (4 heads).
Each core is fully independent (data + head parallel, no collectives).
The host pre-transposes x[n] and the per-core W slices (as bf16);
results are sliced back into out[n, :, 256g:256g+256].

Per-core device program (all matmuls bf16 inputs, f32 PSUM):
  1) kq projection: zT e-tiles [128, seq] = matmul(lhsT=WkqT tile,
     rhs=xT tile); per-partition bias added on the DVE evacuation to
     bf16. Each e-tile holds an even/odd head pair on partitions
     0:64/64:128.
  2) v natural [seq, 4*64] = matmul(lhsT=xT tile, rhs=WvT); bias via
     DVE tensor_tensor; stored bf16 as [128, ktile, head, 65] with a
     ones column for the fused softmax denominator.
  3) Scores per (q-block 512, head pair), k-tiles processed in PAIRS:
     4 matmuls (2 heads x 2 kt) into one 4-bank PSUM tile, one ACT exp
     (scale=1/32, no max subtraction needed) evacuates all 4 quadrants
     to bf16 pt [128, head, kt, q]. Diagonal-pair odd k-tiles widen
     their window 128 left so the pair shares one window (and the
     sub-diagonal region holds real exp values, then zeros). Causal
     masking is one gpsimd affine_select per diagonal pair covering
     [128, 2 heads, 2 kt, w] via pattern coefficient -128 on the kt
     dim.
  4) PV in natural layout: out[q,:] accumulates in PSUM [128, 65] per
     (head, q-tile 128) over k-tiles: matmul(lhsT=pt tile [keys, q],
     rhs=[V|1] [keys, 65]). No PE transposes. DVE reciprocal of col 64
     + tensor_scalar_mul -> normalized rows straight into osb.

Timing (instruction cost model; HW NTFF profiling unavailable through
this axon bridge): see test.py output. Numerical results ARE from
real TRN2 hardware.

_split_matmul_waits() is a required legalization for this compiler
build: every engine instruction may carry at most one semaphore wait.
"""

import numpy as np

import concourse.bass as bass
import concourse.mybir as mybir
import concourse.tile as tile
from concourse.bass_utils import run_bass_kernel_spmd

F32 = mybir.dt.float32
BF = mybir.dt.bfloat16
F8 = mybir.dt.float8e4

# fp8(e4m3) q/k + DoubleRow perf mode halves the score-matmul PE time;
# adds ~1.25e-2 relative error (gate is 2e-2). P/V stay bf16.
FP8_SCORES = True

N, S, D = 2, 2048, 1024
H, HD = 16, 64
P = 128
QB = 512                 # q block (free dim per matmul)
NQB = S // QB            # 4
NKT = S // P             # 16 k tiles
ND = D // P              # 8 contraction tiles
NHC = 4                  # heads per core
EKQ = 2 * NHC * HD       # 512 = k+q rows per core
EV = NHC * HD            # 256 = v rows per core
SCALE = 1.0 / 32.0       # 1/sqrt(1024)

AF = mybir.ActivationFunctionType
ALU = mybir.AluOpType


def _split_matmul_waits(nc):
    """Move extra semaphore waits onto preceding same-engine NOPs.

    The walrus codegen for self-loading matmuls folds waits into the
    LDWEIGHTS struct, which has room for a single sync-wait command;
    sequencer NOPs on the same engine execute in program order, so
    hoisting each wait onto its own NOP is semantics-preserving.
    """
    import bass_rust

    moved = 0
    for bb in nc.main_func.blocks:
        out = []
        for ins in bb.instructions:
            si = ins.sync_info
            keep = 0 if isinstance(ins, bass_rust.InstMatmult) else 1
            if (
                not isinstance(ins, bass_rust.InstNoOp)
                and si is not None
                and len(si.on_wait) > keep
            ):
                hoist = si.on_wait[keep:] if keep else si.on_wait
                for j, w in enumerate(hoist):
                    out.append(
                        bass_rust.InstNoOp(
                            name=f"{ins.name}-hw{j}",
                            engine=ins.engine,
                            sync_info=mybir.SyncInfo(on_wait=[w], on_update=[]),
                        )
                    )
                    moved += 1
                ins.sync_info = mybir.SyncInfo(
                    on_wait=list(si.on_wait[:keep]), on_update=list(si.on_update)
                )
            out.append(ins)
        bb.instructions[:] = out
    return moved


def build_nc(split_waits=True):
    nc = bass.Bass()

    xT = nc.dram_tensor("xT", [D, S], BF, kind="ExternalInput")
    wkq = nc.dram_tensor("wkq", [D, EKQ], BF, kind="ExternalInput")
    wv = nc.dram_tensor("wv", [D, EV], BF, kind="ExternalInput")
    bkq = nc.dram_tensor("bkq", [P, 4], F32, kind="ExternalInput")
    bv = nc.dram_tensor("bv", [1, EV], F32, kind="ExternalInput")
    o = nc.dram_tensor("o", [S, EV], F32, kind="ExternalOutput")

    xT_v = xT.rearrange("(dt p) s -> p dt s", p=P)       # [128, 8, 2048]
    wkq_v = wkq.rearrange("(dt p) e -> p dt e", p=P)     # [128, 8, 512]
    wv_v = wv.rearrange("(dt p) e -> p dt e", p=P)       # [128, 8, 256]
    o_v = o.rearrange("(qt p) c -> p qt c", p=P)         # [128, 16, 256]

    with tile.TileContext(nc) as tc:
        with (
            tc.tile_pool(name="const", bufs=1) as const,
            tc.tile_pool(name="big", bufs=1) as big,
            tc.tile_pool(name="xpool", bufs=2) as xpool,
            tc.tile_pool(name="work", bufs=2) as work,
            tc.tile_pool(name="opool", bufs=2) as opool,
            tc.tile_pool(name="proj_ps", bufs=2, space="PSUM") as proj_ps,
            tc.tile_pool(name="st_ps", bufs=2, space="PSUM") as st_ps,
            tc.tile_pool(name="pv_ps", bufs=2, space="PSUM") as pv_ps,
        ):
            # ---- constants ----
            onesb = const.tile([P, 1], BF)
            nc.vector.memset(onesb, 1.0)
            # warm the ACT exp table while DMAs run
            dummy = const.tile([1, 2], F32)
            nc.gpsimd.memset(dummy, 0.0)
            nc.scalar.activation(dummy, dummy, AF.Exp)
            # diagonal causal mask for the odd head: mask[p, j] = (j >= p)
            mask_sb = const.tile([P, QB], BF)
            nc.gpsimd.affine_select(
                out=mask_sb,
                in_=onesb.to_broadcast((P, QB)),
                compare_op=ALU.is_ge, fill=0.0,
                base=0, channel_multiplier=-1,
                pattern=[[1, QB]],
            )

            # ---- batched input DMAs, critical-path first ----
            # wkq + x0 gate the prologue kq-projection (whose evacuation
            # gates the first exp); everything else follows.
            wkq_sb = const.tile([P, ND, EKQ], BF)
            xqbs = []
            for qb in range(NQB):
                xq = xpool.tile([P, ND, QB], BF, tag=f"xqb{qb}", bufs=1,
                                name=f"xqb{qb}")
                xqbs.append(xq)
            # dt-halves so the first projection chains start after half
            # the data has landed
            nc.sync.dma_start(wkq_sb[:, 0:4], wkq_v[:, 0:4])
            nc.sync.dma_start(xqbs[0][:, 0:4], xT_v[:, 0:4, 0:QB])
            nc.sync.dma_start(wkq_sb[:, 4:8], wkq_v[:, 4:8])
            nc.sync.dma_start(xqbs[0][:, 4:8], xT_v[:, 4:8, 0:QB])
            bkq_sb = const.tile([P, 4], F32)
            nc.sync.dma_start(bkq_sb, bkq[:, :])
            wv_sb = const.tile([P, ND, EV], BF)
            nc.sync.dma_start(wv_sb, wv_v[:, :])
            bvb = const.tile([P, EV], F32)
            nc.sync.dma_start(bvb, bv[:, :].partition_broadcast(P))
            # x blocks 2,3 are prefetched inside the unit loop so the SP
            # queue and DMA engines stay clear for the qb0 re-layouts
            nc.sync.dma_start(xqbs[1], xT_v[:, :, QB:2 * QB])

            # ---- persistent state ----
            # zT for k,q: e-tiles 0,1 = [k_h0;k_h1],[k_h2;k_h3]; 2,3 = q same
            if FP8_SCORES:
                # [p, kq, hp, s] fp8; kq8 is the DoubleRow re-layout
                # [hl*32+p, kq, hp, g, s] with hd split as g*32+p
                zkq = big.tile([P, 2, 2, S], F8)
                kq8 = big.tile([HD, 2, 2, 2, S], F8)
            else:
                zkq = big.tile([P, 4, S], BF)
            # v natural + ones column: [p, ktile, head, 65]
            vsb = big.tile([P, NKT, NHC, HD + 1], BF)
            nc.vector.tensor_copy(
                vsb[:, :, :, HD:HD + 1],
                onesb[:, :, None].to_broadcast((P, NKT, NHC, 1)),
            )  # ones column for the fused sum(exp) row
            # exp(S^T) per (q-block, head pair) unit: [p, head, ktile, q],
            # double-buffered so a unit's PV chains can drain while the
            # next unit's exps write the other buffer
            pta = big.tile([P, 2, NKT, QB], BF)
            ptb = big.tile([P, 2, NKT, QB], BF)
            pts = [pta, ptb]

            proj_state = {}

            def proj_kq_half(qb, xqb, t, half, evac_act=False):
                # ---- projection: zT for one k/q e-tile, dt-half granule
                # (keeps filler chunks under ~1us so the exp stream never
                # waits long for the next score matmul) ----
                if half == 0:
                    pzp = proj_ps.tile([P, QB], F32, tag="projps",
                                       name=f"pzp{qb}_{t}")
                    proj_state[(qb, t)] = pzp
                else:
                    pzp = proj_state.pop((qb, t))
                for dt in range(4 * half, 4 * half + 4):
                    nc.tensor.matmul(
                        pzp,
                        lhsT=(wkq_sb[:, dt, t * P:(t + 1) * P]),
                        rhs=(xqb[:, dt, :]),
                        start=(dt == 0), stop=(dt == ND - 1),
                    )
                if half == 0:
                    return
                if FP8_SCORES:
                    out = zkq[:, t // 2, t % 2, qb * QB:(qb + 1) * QB]
                else:
                    out = zkq[:, t, qb * QB:(qb + 1) * QB]
                if evac_act:
                    # ACT is idle in the early units; evacuating there
                    # keeps the DVE evac stream from pacing the PE proj
                    # chains (proj_ps ring WAR)
                    nc.scalar.activation(
                        out, pzp, AF.Identity, bias=bkq_sb[:, t:t + 1]
                    )
                else:
                    nc.vector.tensor_scalar_add(out, pzp, bkq_sb[:, t:t + 1])
                if FP8_SCORES and t >= 2:
                    # Both e-tiles of head pair hp=t%2 are now evacuated:
                    # DoubleRow re-layout [hl*64+g*32+p -> hl*32+p, g] via
                    # SBUF->SBUF DMAs (SP queue; program order puts them
                    # right after the evacuations).
                    hp = t % 2
                    qw = slice(qb * QB, (qb + 1) * QB)
                    for hl in range(2):
                        for g in range(2):
                            nc.sync.dma_start(
                                kq8[hl * 32:(hl + 1) * 32, :, hp, g, qw],
                                zkq[hl * HD + g * 32:hl * HD + g * 32 + 32,
                                    :, hp, qw],
                            )

            def proj_kq(qb, xqb, tiles, evac_act=False):
                for t in tiles:
                    for half in range(2):
                        proj_kq_half(qb, xqb, t, half, evac_act=evac_act)

            def proj_v1(qb, xqb, qt4):
                # ---- projection: v natural for one q-tile of 128 ----
                qt = qb * 4 + qt4
                pvp = proj_ps.tile([P, QB], F32, tag="projps")
                for dt in range(ND):
                    nc.tensor.matmul(
                        pvp[:, :EV],
                        lhsT=(xqb[:, dt, qt4 * P:(qt4 + 1) * P]),
                        rhs=(wv_sb[:, dt, :]),
                        start=(dt == 0), stop=(dt == ND - 1),
                    )
                nc.vector.tensor_tensor(
                    vsb[:, qt, :, 0:HD],
                    pvp[:, :EV].rearrange("p (h d) -> p h d", d=HD),
                    bvb.rearrange("p (h d) -> p h d", d=HD),
                    mybir.AluOpType.add,
                )

            def attn_scores(qb, hp, pt, fillers=(), direct=False):
                # ---- scores + exp for this q block, head pair hp ----
                # Per k-tile: 2 matmuls (even/odd head, PE row-packed at
                # partition bases 0/64) into a 2-bank PSUM tile; one ACT
                # exp evacuates both heads to bf16 pt. The exp chain paces
                # this unit, so `fillers` (next projections + previous
                # unit's PV chains, ~1-3us of PE work each) are drained
                # between k-tiles to keep PE busy during the per-tile exp
                # handoff.
                kt_t = hp
                qt_t = 2 + hp
                kts = list(range(4 * qb, 4 * qb + 4)) + list(range(4 * qb))
                fill = list(fillers)
                nf = 0
                for i, kt in enumerate(kts):
                    r = kt - 4 * qb
                    diag = 0 <= r < 4
                    off = P * r if diag else 0
                    w = QB - off
                    stp = st_ps.tile([P, 2, QB], F32, tag="st")
                    for hl in range(2):
                        if FP8_SCORES and direct and diag:
                            # straight from zkq (2x the PE rows of the
                            # DoubleRow path but no re-layout dependency;
                            # used for the first unit's critical path)
                            base = HD * hl
                            nc.tensor.matmul(
                                stp[:, hl, off:QB],
                                lhsT=zkq[base:base + HD, 0, hp,
                                         kt * P:(kt + 1) * P],
                                rhs=zkq[base:base + HD, 1, hp,
                                        qb * QB + off:(qb + 1) * QB],
                                start=True, stop=True,
                            )
                        elif FP8_SCORES:
                            nc.tensor.matmul(
                                stp[:, hl, off:QB],
                                lhsT=kq8[hl * 32:(hl + 1) * 32, 0, hp, :,
                                         kt * P:(kt + 1) * P],
                                rhs=kq8[hl * 32:(hl + 1) * 32, 1, hp, :,
                                        qb * QB + off:(qb + 1) * QB],
                                start=True, stop=True,
                                perf_mode=mybir.MatmulPerfMode.DoubleRow,
                            )
                        else:
                            base = HD * hl
                            nc.tensor.matmul(
                                stp[:, hl, off:QB],
                                lhsT=zkq[base:base + HD, kt_t,
                                         kt * P:(kt + 1) * P],
                                rhs=zkq[base:base + HD, qt_t,
                                        qb * QB + off:(qb + 1) * QB],
                                start=True, stop=True,
                            )
                    nc.scalar.activation(
                        pt[:, :, kt, off:QB],
                        stp[:, :, off:QB],
                        AF.Exp, scale=SCALE,
                    )
                    if diag:
                        # causal: keep q >= key (iota = jq - p >= 0)
                        nc.gpsimd.affine_select(
                            out=pt[:, 0, kt, off:QB],
                            in_=pt[:, 0, kt, off:QB],
                            compare_op=ALU.is_ge, fill=0.0,
                            base=0, channel_multiplier=-1,
                            pattern=[[1, w]],
                        )
                        nc.vector.tensor_mul(
                            out=pt[:, 1, kt, off:QB],
                            in0=pt[:, 1, kt, off:QB],
                            in1=mask_sb[:, 0:w],
                        )
                    # drain fillers evenly across the remaining k-tiles
                    want = (len(fill) * (i + 1)) // len(kts)
                    while nf < want:
                        fill[nf]()
                        nf += 1
                while nf < len(fill):
                    fill[nf]()
                    nf += 1

            def pv_chunk(qb, hp, qt4, hl, osb, pt, mul_act=False):
                # ---- PV in natural layout: one (q-tile of 128, head) ----
                nkt_q = 4 * qb + qt4 + 1
                # kt order matches the unit's exp completion order
                kts = list(range(4 * qb, nkt_q)) + list(range(0, 4 * qb))
                pvo = pv_ps.tile([P, HD + 1], F32, tag="pv", name="pvo")
                for i, kt in enumerate(kts):
                    nc.tensor.matmul(
                        pvo,
                        lhsT=pt[:, hl, kt, qt4 * P:(qt4 + 1) * P],
                        rhs=vsb[:, kt, 2 * hp + hl, :],
                        start=(i == 0), stop=(i == nkt_q - 1),
                    )
                h = 2 * hp + hl
                rs = work.tile([P, 1], F32, tag="rs")
                nc.vector.reciprocal(rs, pvo[:, HD:HD + 1])
                if mul_act:
                    # ACT-side normalize (exp stream is drained by now)
                    nc.scalar.mul(
                        osb[:, qt4, HD * h:HD * (h + 1)], pvo[:, 0:HD], rs
                    )
                else:
                    nc.vector.tensor_scalar_mul(
                        osb[:, qt4, HD * h:HD * (h + 1)],
                        pvo[:, 0:HD], rs,
                    )

            # ---- global schedule ----
            # Unit order keeps the biggest (most exp-bound) units in the
            # middle where deferred projection work exists to fill PE
            # stalls, and ends on the smallest unit. Each unit's PV
            # chains drain as fillers inside the NEXT unit's score
            # stream (pt is double-buffered), and each unit also carries
            # the projection chunks needed exactly one unit later.
            # Scores of unit (qb, hp) read zkq k-columns of ALL blocks
            # <= qb for that head pair's e-tiles, so every proj_kq chunk
            # must land before its first reader. Projection work is
            # pushed as LATE as precedence allows so the exp stream (the
            # global pacer) starts early and PE has filler work in the
            # exp-bound late units. K-fillers inside a unit that reads
            # them are only safe when they drain before the history
            # k-tiles that need them (even spread puts the first fillers
            # right after the 4 diagonal k-tiles, and relayout latency is
            # covered by the >=4-tile gap to the first reader).
            def K(qb2, tiles, act=False):
                return [
                    (lambda t=t, h=h, q=qb2:
                     proj_kq_half(q, xqbs[q], t, h, evac_act=act))
                    for t in tiles for h in range(2)
                ]

            def V(qb2):
                return [
                    (lambda q4=q4, q=qb2: proj_v1(q, xqbs[q], q4))
                    for q4 in range(4)
                ]

            units = [(0, 0), (1, 0), (2, 0), (3, 0),
                     (3, 1), (2, 1), (1, 1), (0, 1)]
            proj_fill = {
                (0, 0): K(1, (0, 2), act=True),
                (1, 0): K(2, (0, 2)) + V(0),
                (2, 0): K(3, (0, 2)) + K(0, (1, 3)) + V(1),
                (3, 0): K(3, (1, 3)) + K(1, (1, 3)) + V(2),
                (3, 1): K(2, (1, 3)) + V(3),
                (2, 1): [],
                (1, 1): [],
                (0, 1): [],
            }
            # prologue: ONLY block 0's hp0 k/q e-tiles (they gate the
            # first exp); dt-halves interleaved to match the split DMAs;
            # evacuate on ACT which is idle here
            proj_kq_half(0, xqbs[0], 0, 0, evac_act=True)
            proj_kq_half(0, xqbs[0], 2, 0, evac_act=True)
            proj_kq_half(0, xqbs[0], 0, 1, evac_act=True)
            proj_kq_half(0, xqbs[0], 2, 1, evac_act=True)

            prefetch = {(0, 0): 2, (1, 0): 3}
            osbs = {}
            prev = None
            for ui, (qb, hp) in enumerate(units):
                pt = pts[ui % 2]
                if (qb, hp) in prefetch:
                    nqb = prefetch[(qb, hp)]
                    nc.sync.dma_start(
                        xqbs[nqb], xT_v[:, :, nqb * QB:(nqb + 1) * QB]
                    )
                if hp == 0:
                    osb_t = opool.tile([P, 4, EV], F32, tag="osb", bufs=4,
                                       name=f"osb{qb}")
                    osbs[qb] = osb_t
                fillers = list(proj_fill[(qb, hp)])
                if prev is not None:
                    pqb, php, ppt = prev
                    fillers += [
                        lambda q4=q4, hl=hl, a=pqb, b=php, c=ppt:
                            pv_chunk(a, b, q4, hl, osbs[a], c)
                        for q4 in range(4) for hl in range(2)
                    ]
                if ui == len(units) - 1:
                    # last unit: its own PV chains interleave with its own
                    # exps. Own chunk for q-tile qt only needs exps up to
                    # kt=qt, so lead each group of 4 with it (the spread
                    # drains 4 fillers per k-tile here); normalize muls
                    # alternate DVE/gpsimd to halve the closing cascade.
                    fillers += [
                        lambda q4=q4, hl=hl:
                            pv_chunk(qb, hp, q4, hl, osbs[qb], pt,
                                     mul_act=False)
                        for q4 in range(4) for hl in range(2)
                    ]
                attn_scores(qb, hp, pt, fillers,
                            direct=(hp == 0 and qb <= 1))
                if prev is not None:
                    # half of osb (one head pair) is complete once the
                    # previous unit's PV chains have drained
                    nc.sync.dma_start(
                        o_v[:, pqb * 4:(pqb + 1) * 4, php * P:(php + 1) * P],
                        osbs[pqb][:, :, php * P:(php + 1) * P],
                    )
                prev = (qb, hp, pt)
            nc.sync.dma_start(o_v[:, 0:4, P:2 * P], osbs[0][:, :, P:2 * P])
    if split_waits:
        _split_matmul_waits(nc)
    return nc


_nc_cache = None


def _get_nc():
    global _nc_cache
    if _nc_cache is None:
        _nc_cache = build_nc()
    return _nc_cache


def make_in_maps(x, W, b):
    import ml_dtypes

    x = np.asarray(x, dtype=np.float32)
    W = np.asarray(W, dtype=np.float32)
    b = np.asarray(b, dtype=np.float32)
    bf = ml_dtypes.bfloat16
    in_maps = []
    xTs = [np.ascontiguousarray(x[n].T.astype(bf)) for n in range(N)]
    for c in range(8):
        n, g = divmod(c, 4)
        rk = slice(256 * g, 256 * g + 256)
        rq = slice(D + 256 * g, D + 256 * g + 256)
        rv = slice(2 * D + 256 * g, 2 * D + 256 * g + 256)
        wkq = np.ascontiguousarray(
            np.concatenate([W[rk], W[rq]], axis=0).T.astype(bf)
        )
        wv = np.ascontiguousarray(W[rv].T.astype(bf))
        bkq = np.ascontiguousarray(
            np.concatenate([b[rk], b[rq]]).reshape(4, P).T
        )
        bv = np.ascontiguousarray(b[rv].reshape(1, EV))
        in_maps.append(
            {"xT": xTs[n], "wkq": wkq, "wv": wv, "bkq": bkq, "bv": bv}
        )
    return in_maps


def run(inputs, **kwargs):
    nc = _get_nc()
    in_maps = make_in_maps(inputs["x"], inputs["W"], inputs["b"])
    res = run_bass_kernel_spmd(nc, in_maps, core_ids=list(range(8)), **kwargs)
    out = np.empty((N, S, D), dtype=np.float32)
    for c in range(8):
        n, g = divmod(c, 4)
        out[n, :, 256 * g:256 * g + 256] = res.results[c]["o"]
    return out, res


def kernel(**inputs):
    out, _ = run(inputs)
    return out


# revision 49
# speedup vs baseline: 1.0010x; 1.0010x over previous
"""Masked multi-head attention (fused QKV) on 8 trn2 NeuronCores.

Problem (full shapes): x [2, 2048, 1024] f32, W [3072, 1024], b [3072].
  z = x @ W.T + b ; k,q,v = split(z) ; heads H=16, hd=64
  out = softmax(causal(q k^T / sqrt(1024))) v   -> [2, 2048, 1024]

Sharding: core c handles batch n=c//4 and head group g=c%4 (4 heads).
Each core is fully independent (data + head parallel, no collectives).
The host pre-transposes x[n] and the per-core W slices (as bf16);
results are sliced back into out[n, :, 256g:256g+256].

Per-core device program (all matmuls bf16 inputs, f32 PSUM):
  1) kq projection: zT e-tiles [128, seq] = matmul(lhsT=WkqT tile,
     rhs=xT tile); per-partition bias added on the DVE evacuation to
     bf16. Each e-tile holds an even/odd head pair on partitions
     0:64/64:128.
  2) v natural [seq, 4*64] = matmul(lhsT=xT tile, rhs=WvT); bias via
     DVE tensor_tensor; stored bf16 as [128, ktile, head, 65] with a
     ones column for the fused softmax denominator.
  3) Scores per (q-block 512, head pair), k-tiles processed in PAIRS:
     4 matmuls (2 heads x 2 kt) into one 4-bank PSUM tile, one ACT exp
     (scale=1/32, no max subtraction needed) evacuates all 4 quadrants
     to bf16 pt [128, head, kt, q]. Diagonal-pair odd k-tiles widen
     their window 128 left so the pair shares one window (and the
     sub-diagonal region holds real exp values, then zeros). Causal
     masking is one gpsimd affine_select per diagonal pair covering
     [128, 2 heads, 2 kt, w] via pattern coefficient -128 on the kt
     dim.
  4) PV in natural layout: out[q,:] accumulates in PSUM [128, 65] per
     (head, q-tile 128) over k-tiles: matmul(lhsT=pt tile [keys, q],
     rhs=[V|1] [keys, 65]). No PE transposes. DVE reciprocal of col 64
     + tensor_scalar_mul -> normalized rows straight into osb.

Timing (instruction cost model; HW NTFF profiling unavailable through
this axon bridge): see test.py output. Numerical results ARE from
real TRN2 hardware.

_split_matmul_waits() is a required legalization for this compiler
build: every engine instruction may carry at most one semaphore wait.
"""

import numpy as np

import concourse.bass as bass
import concourse.mybir as mybir
import concourse.tile as tile
from concourse.bass_utils import run_bass_kernel_spmd

F32 = mybir.dt.float32
BF = mybir.dt.bfloat16
F8 = mybir.dt.float8e4

# fp8(e4m3) q/k + DoubleRow perf mode halves the score-matmul PE time;
# adds ~1.25e-2 relative error (gate is 2e-2). P/V stay bf16.
FP8_SCORES = True

N, S, D = 2, 2048, 1024
H, HD = 16, 64
P = 128
QB = 512                 # q block (free dim per matmul)
NQB = S // QB            # 4
NKT = S // P             # 16 k tiles
ND = D // P              # 8 contraction tiles
NHC = 4                  # heads per core
EKQ = 2 * NHC * HD       # 512 = k+q rows per core
EV = NHC * HD            # 256 = v rows per core
SCALE = 1.0 / 32.0       # 1/sqrt(1024)

AF = mybir.ActivationFunctionType
ALU = mybir.AluOpType


def _split_matmul_waits(nc):
    """Move extra semaphore waits onto preceding same-engine NOPs.

    The walrus codegen for self-loading matmuls folds waits into the
    LDWEIGHTS struct, which has room for a single sync-wait command;
    sequencer NOPs on the same engine execute in program order, so
    hoisting each wait onto its own NOP is semantics-preserving.
    """
    import bass_rust

    moved = 0
    for bb in nc.main_func.blocks:
        out = []
        for ins in bb.instructions:
            si = ins.sync_info
            keep = 0 if isinstance(ins, bass_rust.InstMatmult) else 1
            if (
                not isinstance(ins, bass_rust.InstNoOp)
                and si is not None
                and len(si.on_wait) > keep
            ):
                hoist = si.on_wait[keep:] if keep else si.on_wait
                for j, w in enumerate(hoist):
                    out.append(
                        bass_rust.InstNoOp(
                            name=f"{ins.name}-hw{j}",
                            engine=ins.engine,
                            sync_info=mybir.SyncInfo(on_wait=[w], on_update=[]),
                        )
                    )
                    moved += 1
                ins.sync_info = mybir.SyncInfo(
                    on_wait=list(si.on_wait[:keep]), on_update=list(si.on_update)
                )
            out.append(ins)
        bb.instructions[:] = out
    return moved


def build_nc(split_waits=True):
    nc = bass.Bass()

    xT = nc.dram_tensor("xT", [D, S], BF, kind="ExternalInput")
    wkq = nc.dram_tensor("wkq", [D, EKQ], BF, kind="ExternalInput")
    wv = nc.dram_tensor("wv", [D, EV], BF, kind="ExternalInput")
    bkq = nc.dram_tensor("bkq", [P, 4], F32, kind="ExternalInput")
    bv = nc.dram_tensor("bv", [1, EV], F32, kind="ExternalInput")
    o = nc.dram_tensor("o", [S, EV], F32, kind="ExternalOutput")

    xT_v = xT.rearrange("(dt p) s -> p dt s", p=P)       # [128, 8, 2048]
    wkq_v = wkq.rearrange("(dt p) e -> p dt e", p=P)     # [128, 8, 512]
    wv_v = wv.rearrange("(dt p) e -> p dt e", p=P)       # [128, 8, 256]
    o_v = o.rearrange("(qt p) c -> p qt c", p=P)         # [128, 16, 256]

    with tile.TileContext(nc) as tc:
        with (
            tc.tile_pool(name="const", bufs=1) as const,
            tc.tile_pool(name="big", bufs=1) as big,
            tc.tile_pool(name="xpool", bufs=2) as xpool,
            tc.tile_pool(name="work", bufs=2) as work,
            tc.tile_pool(name="opool", bufs=2) as opool,
            tc.tile_pool(name="proj_ps", bufs=2, space="PSUM") as proj_ps,
            tc.tile_pool(name="st_ps", bufs=2, space="PSUM") as st_ps,
            tc.tile_pool(name="pv_ps", bufs=2, space="PSUM") as pv_ps,
        ):
            # ---- constants ----
            onesb = const.tile([P, 1], BF)
            nc.vector.memset(onesb, 1.0)
            # warm the ACT exp table while DMAs run
            dummy = const.tile([1, 2], F32)
            nc.gpsimd.memset(dummy, 0.0)
            nc.scalar.activation(dummy, dummy, AF.Exp)
            # diagonal causal mask for the odd head: mask[p, j] = (j >= p)
            mask_sb = const.tile([P, QB], BF)
            nc.gpsimd.affine_select(
                out=mask_sb,
                in_=onesb.to_broadcast((P, QB)),
                compare_op=ALU.is_ge, fill=0.0,
                base=0, channel_multiplier=-1,
                pattern=[[1, QB]],
            )

            # ---- batched input DMAs, critical-path first ----
            # wkq + x0 gate the prologue kq-projection (whose evacuation
            # gates the first exp); everything else follows.
            wkq_sb = const.tile([P, ND, EKQ], BF)
            xqbs = []
            for qb in range(NQB):
                xq = xpool.tile([P, ND, QB], BF, tag=f"xqb{qb}", bufs=1,
                                name=f"xqb{qb}")
                xqbs.append(xq)
            # dt-halves so the first projection chains start after half
            # the data has landed
            nc.sync.dma_start(wkq_sb[:, 0:4], wkq_v[:, 0:4])
            nc.sync.dma_start(xqbs[0][:, 0:4], xT_v[:, 0:4, 0:QB])
            nc.sync.dma_start(wkq_sb[:, 4:8], wkq_v[:, 4:8])
            nc.sync.dma_start(xqbs[0][:, 4:8], xT_v[:, 4:8, 0:QB])
            bkq_sb = const.tile([P, 4], F32)
            nc.sync.dma_start(bkq_sb, bkq[:, :])
            wv_sb = const.tile([P, ND, EV], BF)
            nc.sync.dma_start(wv_sb, wv_v[:, :])
            bvb = const.tile([P, EV], F32)
            nc.sync.dma_start(bvb, bv[:, :].partition_broadcast(P))
            # x blocks 2,3 are prefetched inside the unit loop so the SP
            # queue and DMA engines stay clear for the qb0 re-layouts
            nc.sync.dma_start(xqbs[1], xT_v[:, :, QB:2 * QB])

            # ---- persistent state ----
            # zT for k,q: e-tiles 0,1 = [k_h0;k_h1],[k_h2;k_h3]; 2,3 = q same
            if FP8_SCORES:
                # [p, kq, hp, s] fp8; kq8 is the DoubleRow re-layout
                # [hl*32+p, kq, hp, g, s] with hd split as g*32+p
                zkq = big.tile([P, 2, 2, S], F8)
                kq8 = big.tile([HD, 2, 2, 2, S], F8)
            else:
                zkq = big.tile([P, 4, S], BF)
            # v natural + ones column: [p, ktile, head, 65]
            vsb = big.tile([P, NKT, NHC, HD + 1], BF)
            nc.vector.tensor_copy(
                vsb[:, :, :, HD:HD + 1],
                onesb[:, :, None].to_broadcast((P, NKT, NHC, 1)),
            )  # ones column for the fused sum(exp) row
            # exp(S^T) per (q-block, head pair) unit: [p, head, ktile, q],
            # double-buffered so a unit's PV chains can drain while the
            # next unit's exps write the other buffer
            pta = big.tile([P, 2, NKT, QB], BF)
            ptb = big.tile([P, 2, NKT, QB], BF)
            pts = [pta, ptb]

            proj_state = {}

            def proj_kq_half(qb, xqb, t, half, evac_act=False):
                # ---- projection: zT for one k/q e-tile, dt-half granule
                # (keeps filler chunks under ~1us so the exp stream never
                # waits long for the next score matmul) ----
                if half == 0:
                    pzp = proj_ps.tile([P, QB], F32, tag="projps",
                                       name=f"pzp{qb}_{t}")
                    proj_state[(qb, t)] = pzp
                else:
                    pzp = proj_state.pop((qb, t))
                for dt in range(4 * half, 4 * half + 4):
                    nc.tensor.matmul(
                        pzp,
                        lhsT=(wkq_sb[:, dt, t * P:(t + 1) * P]),
                        rhs=(xqb[:, dt, :]),
                        start=(dt == 0), stop=(dt == ND - 1),
                    )
                if half == 0:
                    return
                if FP8_SCORES:
                    out = zkq[:, t // 2, t % 2, qb * QB:(qb + 1) * QB]
                else:
                    out = zkq[:, t, qb * QB:(qb + 1) * QB]
                if evac_act:
                    # ACT is idle in the early units; evacuating there
                    # keeps the DVE evac stream from pacing the PE proj
                    # chains (proj_ps ring WAR)
                    nc.scalar.activation(
                        out, pzp, AF.Identity, bias=bkq_sb[:, t:t + 1]
                    )
                else:
                    nc.vector.tensor_scalar_add(out, pzp, bkq_sb[:, t:t + 1])
                if FP8_SCORES and t >= 2:
                    # Both e-tiles of head pair hp=t%2 are now evacuated:
                    # DoubleRow re-layout [hl*64+g*32+p -> hl*32+p, g] via
                    # SBUF->SBUF DMAs (SP queue; program order puts them
                    # right after the evacuations).
                    hp = t % 2
                    qw = slice(qb * QB, (qb + 1) * QB)
                    for hl in range(2):
                        for g in range(2):
                            nc.sync.dma_start(
                                kq8[hl * 32:(hl + 1) * 32, :, hp, g, qw],
                                zkq[hl * HD + g * 32:hl * HD + g * 32 + 32,
                                    :, hp, qw],
                            )

            def proj_kq(qb, xqb, tiles, evac_act=False):
                for t in tiles:
                    for half in range(2):
                        proj_kq_half(qb, xqb, t, half, evac_act=evac_act)

            def proj_v1(qb, xqb, qt4):
                # ---- projection: v natural for one q-tile of 128 ----
                qt = qb * 4 + qt4
                pvp = proj_ps.tile([P, QB], F32, tag="projps")
                for dt in range(ND):
                    nc.tensor.matmul(
                        pvp[:, :EV],
                        lhsT=(xqb[:, dt, qt4 * P:(qt4 + 1) * P]),
                        rhs=(wv_sb[:, dt, :]),
                        start=(dt == 0), stop=(dt == ND - 1),
                    )
                nc.vector.tensor_tensor(
                    vsb[:, qt, :, 0:HD],
                    pvp[:, :EV].rearrange("p (h d) -> p h d", d=HD),
                    bvb.rearrange("p (h d) -> p h d", d=HD),
                    mybir.AluOpType.add,
                )

            def attn_scores(qb, hp, pt, fillers=(), direct=False):
                # ---- scores + exp for this q block, head pair hp ----
                # Per k-tile: 2 matmuls (even/odd head, PE row-packed at
                # partition bases 0/64) into a 2-bank PSUM tile; one ACT
                # exp evacuates both heads to bf16 pt. The exp chain paces
                # this unit, so `fillers` (next projections + previous
                # unit's PV chains, ~1-3us of PE work each) are drained
                # between k-tiles to keep PE busy during the per-tile exp
                # handoff.
                kt_t = hp
                qt_t = 2 + hp
                kts = list(range(4 * qb, 4 * qb + 4)) + list(range(4 * qb))
                fill = list(fillers)
                nf = 0
                for i, kt in enumerate(kts):
                    r = kt - 4 * qb
                    diag = 0 <= r < 4
                    off = P * r if diag else 0
                    w = QB - off
                    stp = st_ps.tile([P, 2, QB], F32, tag="st")
                    for hl in range(2):
                        if FP8_SCORES and direct and diag:
                            # straight from zkq (2x the PE rows of the
                            # DoubleRow path but no re-layout dependency;
                            # used for the first unit's critical path)
                            base = HD * hl
                            nc.tensor.matmul(
                                stp[:, hl, off:QB],
                                lhsT=zkq[base:base + HD, 0, hp,
                                         kt * P:(kt + 1) * P],
                                rhs=zkq[base:base + HD, 1, hp,
                                        qb * QB + off:(qb + 1) * QB],
                                start=True, stop=True,
                            )
                        elif FP8_SCORES:
                            nc.tensor.matmul(
                                stp[:, hl, off:QB],
                                lhsT=kq8[hl * 32:(hl + 1) * 32, 0, hp, :,
                                         kt * P:(kt + 1) * P],
                                rhs=kq8[hl * 32:(hl + 1) * 32, 1, hp, :,
                                        qb * QB + off:(qb + 1) * QB],
                                start=True, stop=True,
                                perf_mode=mybir.MatmulPerfMode.DoubleRow,
                            )
                        else:
                            base = HD * hl
                            nc.tensor.matmul(
                                stp[:, hl, off:QB],
                                lhsT=zkq[base:base + HD, kt_t,
                                         kt * P:(kt + 1) * P],
                                rhs=zkq[base:base + HD, qt_t,
                                        qb * QB + off:(qb + 1) * QB],
                                start=True, stop=True,
                            )
                    nc.scalar.activation(
                        pt[:, :, kt, off:QB],
                        stp[:, :, off:QB],
                        AF.Exp, scale=SCALE,
                    )
                    if diag:
                        # causal: keep q >= key (iota = jq - p >= 0)
                        nc.gpsimd.affine_select(
                            out=pt[:, 0, kt, off:QB],
                            in_=pt[:, 0, kt, off:QB],
                            compare_op=ALU.is_ge, fill=0.0,
                            base=0, channel_multiplier=-1,
                            pattern=[[1, w]],
                        )
                        nc.vector.tensor_mul(
                            out=pt[:, 1, kt, off:QB],
                            in0=pt[:, 1, kt, off:QB],
                            in1=mask_sb[:, 0:w],
                        )
                    # drain fillers evenly across the remaining k-tiles
                    want = (len(fill) * (i + 1)) // len(kts)
                    while nf < want:
                        fill[nf]()
                        nf += 1
                while nf < len(fill):
                    fill[nf]()
                    nf += 1

            def pv_chunk(qb, hp, qt4, hl, osb, pt, mul_act=False):
                # ---- PV in natural layout: one (q-tile of 128, head) ----
                nkt_q = 4 * qb + qt4 + 1
                # kt order matches the unit's exp completion order
                kts = list(range(4 * qb, nkt_q)) + list(range(0, 4 * qb))
                pvo = pv_ps.tile([P, HD + 1], F32, tag="pv", name="pvo")
                for i, kt in enumerate(kts):
                    nc.tensor.matmul(
                        pvo,
                        lhsT=pt[:, hl, kt, qt4 * P:(qt4 + 1) * P],
                        rhs=vsb[:, kt, 2 * hp + hl, :],
                        start=(i == 0), stop=(i == nkt_q - 1),
                    )
                h = 2 * hp + hl
                rs = work.tile([P, 1], F32, tag="rs")
                nc.vector.reciprocal(rs, pvo[:, HD:HD + 1])
                if mul_act:
                    # ACT-side normalize (exp stream is drained by now)
                    nc.scalar.mul(
                        osb[:, qt4, HD * h:HD * (h + 1)], pvo[:, 0:HD], rs
                    )
                else:
                    nc.vector.tensor_scalar_mul(
                        osb[:, qt4, HD * h:HD * (h + 1)],
                        pvo[:, 0:HD], rs,
                    )

            # ---- global schedule ----
            # Unit order keeps the biggest (most exp-bound) units in the
            # middle where deferred projection work exists to fill PE
            # stalls, and ends on the smallest unit. Each unit's PV
            # chains drain as fillers inside the NEXT unit's score
            # stream (pt is double-buffered), and each unit also carries
            # the projection chunks needed exactly one unit later.
            # Scores of unit (qb, hp) read zkq k-columns of ALL blocks
            # <= qb for that head pair's e-tiles, so every proj_kq chunk
            # must land before its first reader. Projection work is
            # pushed as LATE as precedence allows so the exp stream (the
            # global pacer) starts early and PE has filler work in the
            # exp-bound late units. K-fillers inside a unit that reads
            # them are only safe when they drain before the history
            # k-tiles that need them (even spread puts the first fillers
            # right after the 4 diagonal k-tiles, and relayout latency is
            # covered by the >=4-tile gap to the first reader).
            def K(qb2, tiles, act=False):
                return [
                    (lambda t=t, h=h, q=qb2:
                     proj_kq_half(q, xqbs[q], t, h, evac_act=act))
                    for t in tiles for h in range(2)
                ]

            def V(qb2):
                return [
                    (lambda q4=q4, q=qb2: proj_v1(q, xqbs[q], q4))
                    for q4 in range(4)
                ]

            units = [(0, 0), (1, 0), (2, 0), (3, 0),
                     (3, 1), (2, 1), (1, 1), (0, 1)]
            proj_fill = {
                (0, 0): K(1, (0, 2)),
                (1, 0): K(2, (0, 2)) + V(0),
                (2, 0): K(3, (0, 2)) + K(0, (1, 3)) + V(1),
                (3, 0): K(3, (1, 3)) + K(1, (1, 3)) + V(2),
                (3, 1): K(2, (1, 3)) + V(3),
                (2, 1): [],
                (1, 1): [],
                (0, 1): [],
            }
            # prologue: ONLY block 0's hp0 k/q e-tiles (they gate the
            # first exp); dt-halves interleaved to match the split DMAs;
            # evacuate on ACT which is idle here
            proj_kq_half(0, xqbs[0], 0, 0, evac_act=True)
            proj_kq_half(0, xqbs[0], 2, 0, evac_act=True)
            proj_kq_half(0, xqbs[0], 0, 1, evac_act=True)
            proj_kq_half(0, xqbs[0], 2, 1, evac_act=True)

            prefetch = {(0, 0): 2, (1, 0): 3}
            osbs = {}
            prev = None
            for ui, (qb, hp) in enumerate(units):
                pt = pts[ui % 2]
                if (qb, hp) in prefetch:
                    nqb = prefetch[(qb, hp)]
                    nc.sync.dma_start(
                        xqbs[nqb], xT_v[:, :, nqb * QB:(nqb + 1) * QB]
                    )
                if hp == 0:
                    osb_t = opool.tile([P, 4, EV], F32, tag="osb", bufs=4,
                                       name=f"osb{qb}")
                    osbs[qb] = osb_t
                fillers = list(proj_fill[(qb, hp)])
                if prev is not None:
                    pqb, php, ppt = prev
                    fillers += [
                        lambda q4=q4, hl=hl, a=pqb, b=php, c=ppt:
                            pv_chunk(a, b, q4, hl, osbs[a], c)
                        for q4 in range(4) for hl in range(2)
                    ]
                if ui == len(units) - 1:
                    # last unit: its own PV chains interleave with its own
                    # exps. Own chunk for q-tile qt only needs exps up to
                    # kt=qt, so lead each group of 4 with it (the spread
                    # drains 4 fillers per k-tile here); normalize muls
                    # alternate DVE/gpsimd to halve the closing cascade.
                    fillers += [
                        lambda q4=q4, hl=hl:
                            pv_chunk(qb, hp, q4, hl, osbs[qb], pt,
                                     mul_act=False)
                        for q4 in range(4) for hl in range(2)
                    ]
                attn_scores(qb, hp, pt, fillers,
                            direct=(hp == 0 and qb <= 1))
                if prev is not None:
                    # half of osb (one head pair) is complete once the
                    # previous unit's PV chains have drained
                    nc.sync.dma_start(
                        o_v[:, pqb * 4:(pqb + 1) * 4, php * P:(php + 1) * P],
                        osbs[pqb][:, :, php * P:(php + 1) * P],
                    )
                prev = (qb, hp, pt)
            nc.sync.dma_start(o_v[:, 0:4, P:2 * P], osbs[0][:, :, P:2 * P])
    if split_waits:
        _split_matmul_waits(nc)
    return nc


_nc_cache = None


def _get_nc():
    global _nc_cache
    if _nc_cache is None:
        _nc_cache = build_nc()
    return _nc_cache


def make_in_maps(x, W, b):
    import ml_dtypes

    x = np.asarray(x, dtype=np.float32)
    W = np.asarray(W, dtype=np.float32)
    b = np.asarray(b, dtype=np.float32)
    bf = ml_dtypes.bfloat16
    in_maps = []
    xTs = [np.ascontiguousarray(x[n].T.astype(bf)) for n in range(N)]
    for c in range(8):
        n, g = divmod(c, 4)
        rk = slice(256 * g, 256 * g + 256)
        rq = slice(D + 256 * g, D + 256 * g + 256)
        rv = slice(2 * D + 256 * g, 2 * D + 256 * g + 256)
        wkq = np.ascontiguousarray(
            np.concatenate([W[rk], W[rq]], axis=0).T.astype(bf)
        )
        wv = np.ascontiguousarray(W[rv].T.astype(bf))
        bkq = np.ascontiguousarray(
            np.concatenate([b[rk], b[rq]]).reshape(4, P).T
        )
        bv = np.ascontiguousarray(b[rv].reshape(1, EV))
        in_maps.append(
            {"xT": xTs[n], "wkq": wkq, "wv": wv, "bkq": bkq, "bv": bv}
        )
    return in_maps


def run(inputs, **kwargs):
    nc = _get_nc()
    in_maps = make_in_maps(inputs["x"], inputs["W"], inputs["b"])
    res = run_bass_kernel_spmd(nc, in_maps, core_ids=list(range(8)), **kwargs)
    out = np.empty((N, S, D), dtype=np.float32)
    for c in range(8):
        n, g = divmod(c, 4)
        out[n, :, 256 * g:256 * g + 256] = res.results[c]["o"]
    return out, res


def kernel(**inputs):
    out, _ = run(inputs)
    return out


# revision 50
# speedup vs baseline: 1.0064x; 1.0054x over previous
"""Masked multi-head attention (fused QKV) on 8 trn2 NeuronCores.

Problem (full shapes): x [2, 2048, 1024] f32, W [3072, 1024], b [3072].
  z = x @ W.T + b ; k,q,v = split(z) ; heads H=16, hd=64
  out = softmax(causal(q k^T / sqrt(1024))) v   -> [2, 2048, 1024]

Sharding: core c handles batch n=c//4 and head group g=c%4 (4 heads).
Each core is fully independent (data + head parallel, no collectives).
The host pre-transposes x[n] and the per-core W slices (as bf16);
results are sliced back into out[n, :, 256g:256g+256].

Per-core device program (all matmuls bf16 inputs, f32 PSUM):
  1) kq projection: zT e-tiles [128, seq] = matmul(lhsT=WkqT tile,
     rhs=xT tile); per-partition bias added on the DVE evacuation to
     bf16. Each e-tile holds an even/odd head pair on partitions
     0:64/64:128.
  2) v natural [seq, 4*64] = matmul(lhsT=xT tile, rhs=WvT); bias via
     DVE tensor_tensor; stored bf16 as [128, ktile, head, 65] with a
     ones column for the fused softmax denominator.
  3) Scores per (q-block 512, head pair), k-tiles processed in PAIRS:
     4 matmuls (2 heads x 2 kt) into one 4-bank PSUM tile, one ACT exp
     (scale=1/32, no max subtraction needed) evacuates all 4 quadrants
     to bf16 pt [128, head, kt, q]. Diagonal-pair odd k-tiles widen
     their window 128 left so the pair shares one window (and the
     sub-diagonal region holds real exp values, then zeros). Causal
     masking is one gpsimd affine_select per diagonal pair covering
     [128, 2 heads, 2 kt, w] via pattern coefficient -128 on the kt
     dim.
  4) PV in natural layout: out[q,:] accumulates in PSUM [128, 65] per
     (head, q-tile 128) over k-tiles: matmul(lhsT=pt tile [keys, q],
     rhs=[V|1] [keys, 65]). No PE transposes. DVE reciprocal of col 64
     + tensor_scalar_mul -> normalized rows straight into osb.

Timing (instruction cost model; HW NTFF profiling unavailable through
this axon bridge): see test.py output. Numerical results ARE from
real TRN2 hardware.

_split_matmul_waits() is a required legalization for this compiler
build: every engine instruction may carry at most one semaphore wait.
"""

import numpy as np

import concourse.bass as bass
import concourse.mybir as mybir
import concourse.tile as tile
from concourse.bass_utils import run_bass_kernel_spmd

F32 = mybir.dt.float32
BF = mybir.dt.bfloat16
F8 = mybir.dt.float8e4

# fp8(e4m3) q/k + DoubleRow perf mode halves the score-matmul PE time;
# adds ~1.25e-2 relative error (gate is 2e-2). P/V stay bf16.
FP8_SCORES = True

N, S, D = 2, 2048, 1024
H, HD = 16, 64
P = 128
QB = 512                 # q block (free dim per matmul)
NQB = S // QB            # 4
NKT = S // P             # 16 k tiles
ND = D // P              # 8 contraction tiles
NHC = 4                  # heads per core
EKQ = 2 * NHC * HD       # 512 = k+q rows per core
EV = NHC * HD            # 256 = v rows per core
SCALE = 1.0 / 32.0       # 1/sqrt(1024)

AF = mybir.ActivationFunctionType
ALU = mybir.AluOpType


def _split_matmul_waits(nc):
    """Move extra semaphore waits onto preceding same-engine NOPs.

    The walrus codegen for self-loading matmuls folds waits into the
    LDWEIGHTS struct, which has room for a single sync-wait command;
    sequencer NOPs on the same engine execute in program order, so
    hoisting each wait onto its own NOP is semantics-preserving.
    """
    import bass_rust

    moved = 0
    for bb in nc.main_func.blocks:
        out = []
        for ins in bb.instructions:
            si = ins.sync_info
            keep = 0 if isinstance(ins, bass_rust.InstMatmult) else 1
            if (
                not isinstance(ins, bass_rust.InstNoOp)
                and si is not None
                and len(si.on_wait) > keep
            ):
                hoist = si.on_wait[keep:] if keep else si.on_wait
                for j, w in enumerate(hoist):
                    out.append(
                        bass_rust.InstNoOp(
                            name=f"{ins.name}-hw{j}",
                            engine=ins.engine,
                            sync_info=mybir.SyncInfo(on_wait=[w], on_update=[]),
                        )
                    )
                    moved += 1
                ins.sync_info = mybir.SyncInfo(
                    on_wait=list(si.on_wait[:keep]), on_update=list(si.on_update)
                )
            out.append(ins)
        bb.instructions[:] = out
    return moved


def build_nc(split_waits=True):
    nc = bass.Bass()

    xT = nc.dram_tensor("xT", [D, S], BF, kind="ExternalInput")
    wkq = nc.dram_tensor("wkq", [D, EKQ], BF, kind="ExternalInput")
    wv = nc.dram_tensor("wv", [D, EV], BF, kind="ExternalInput")
    bkq = nc.dram_tensor("bkq", [P, 4], F32, kind="ExternalInput")
    bv = nc.dram_tensor("bv", [1, EV], F32, kind="ExternalInput")
    o = nc.dram_tensor("o", [S, EV], F32, kind="ExternalOutput")

    xT_v = xT.rearrange("(dt p) s -> p dt s", p=P)       # [128, 8, 2048]
    wkq_v = wkq.rearrange("(dt p) e -> p dt e", p=P)     # [128, 8, 512]
    wv_v = wv.rearrange("(dt p) e -> p dt e", p=P)       # [128, 8, 256]
    o_v = o.rearrange("(qt p) c -> p qt c", p=P)         # [128, 16, 256]

    with tile.TileContext(nc) as tc:
        with (
            tc.tile_pool(name="const", bufs=1) as const,
            tc.tile_pool(name="big", bufs=1) as big,
            tc.tile_pool(name="xpool", bufs=2) as xpool,
            tc.tile_pool(name="work", bufs=2) as work,
            tc.tile_pool(name="opool", bufs=2) as opool,
            tc.tile_pool(name="proj_ps", bufs=2, space="PSUM") as proj_ps,
            tc.tile_pool(name="st_ps", bufs=2, space="PSUM") as st_ps,
            tc.tile_pool(name="pv_ps", bufs=2, space="PSUM") as pv_ps,
        ):
            # ---- constants ----
            onesb = const.tile([P, 1], BF)
            nc.vector.memset(onesb, 1.0)
            # warm the ACT exp table while DMAs run
            dummy = const.tile([1, 2], F32)
            nc.gpsimd.memset(dummy, 0.0)
            nc.scalar.activation(dummy, dummy, AF.Exp)
            # diagonal causal mask for the odd head: mask[p, j] = (j >= p)
            mask_sb = const.tile([P, QB], BF)
            nc.gpsimd.affine_select(
                out=mask_sb,
                in_=onesb.to_broadcast((P, QB)),
                compare_op=ALU.is_ge, fill=0.0,
                base=0, channel_multiplier=-1,
                pattern=[[1, QB]],
            )

            # ---- batched input DMAs, critical-path first ----
            # wkq + x0 gate the prologue kq-projection (whose evacuation
            # gates the first exp); everything else follows.
            wkq_sb = const.tile([P, ND, EKQ], BF)
            xqbs = []
            for qb in range(NQB):
                xq = xpool.tile([P, ND, QB], BF, tag=f"xqb{qb}", bufs=1,
                                name=f"xqb{qb}")
                xqbs.append(xq)
            nc.sync.dma_start(wkq_sb, wkq_v[:, :])
            nc.sync.dma_start(xqbs[0], xT_v[:, :, 0:QB])
            bkq_sb = const.tile([P, 4], F32)
            nc.sync.dma_start(bkq_sb, bkq[:, :])
            wv_sb = const.tile([P, ND, EV], BF)
            nc.sync.dma_start(wv_sb, wv_v[:, :])
            bvb = const.tile([P, EV], F32)
            nc.sync.dma_start(bvb, bv[:, :].partition_broadcast(P))
            # x blocks 2,3 are prefetched inside the unit loop so the SP
            # queue and DMA engines stay clear for the qb0 re-layouts
            nc.sync.dma_start(xqbs[1], xT_v[:, :, QB:2 * QB])

            # ---- persistent state ----
            # zT for k,q: e-tiles 0,1 = [k_h0;k_h1],[k_h2;k_h3]; 2,3 = q same
            if FP8_SCORES:
                # [p, kq, hp, s] fp8; kq8 is the DoubleRow re-layout
                # [hl*32+p, kq, hp, g, s] with hd split as g*32+p
                zkq = big.tile([P, 2, 2, S], F8)
                kq8 = big.tile([HD, 2, 2, 2, S], F8)
            else:
                zkq = big.tile([P, 4, S], BF)
            # v natural + ones column: [p, ktile, head, 65]
            vsb = big.tile([P, NKT, NHC, HD + 1], BF)
            nc.vector.tensor_copy(
                vsb[:, :, :, HD:HD + 1],
                onesb[:, :, None].to_broadcast((P, NKT, NHC, 1)),
            )  # ones column for the fused sum(exp) row
            # exp(S^T) per (q-block, head pair) unit: [p, head, ktile, q],
            # double-buffered so a unit's PV chains can drain while the
            # next unit's exps write the other buffer
            pta = big.tile([P, 2, NKT, QB], BF)
            ptb = big.tile([P, 2, NKT, QB], BF)
            pts = [pta, ptb]

            proj_state = {}

            def proj_kq_half(qb, xqb, t, half, evac_act=False):
                # ---- projection: zT for one k/q e-tile, dt-half granule
                # (keeps filler chunks under ~1us so the exp stream never
                # waits long for the next score matmul) ----
                if half == 0:
                    pzp = proj_ps.tile([P, QB], F32, tag="projps",
                                       name=f"pzp{qb}_{t}")
                    proj_state[(qb, t)] = pzp
                else:
                    pzp = proj_state.pop((qb, t))
                for dt in range(4 * half, 4 * half + 4):
                    nc.tensor.matmul(
                        pzp,
                        lhsT=(wkq_sb[:, dt, t * P:(t + 1) * P]),
                        rhs=(xqb[:, dt, :]),
                        start=(dt == 0), stop=(dt == ND - 1),
                    )
                if half == 0:
                    return
                if FP8_SCORES:
                    out = zkq[:, t // 2, t % 2, qb * QB:(qb + 1) * QB]
                else:
                    out = zkq[:, t, qb * QB:(qb + 1) * QB]
                if evac_act:
                    # ACT is idle in the early units; evacuating there
                    # keeps the DVE evac stream from pacing the PE proj
                    # chains (proj_ps ring WAR)
                    nc.scalar.activation(
                        out, pzp, AF.Identity, bias=bkq_sb[:, t:t + 1]
                    )
                else:
                    nc.vector.tensor_scalar_add(out, pzp, bkq_sb[:, t:t + 1])
                if FP8_SCORES and t >= 2:
                    # Both e-tiles of head pair hp=t%2 are now evacuated:
                    # DoubleRow re-layout [hl*64+g*32+p -> hl*32+p, g] via
                    # SBUF->SBUF DMAs (SP queue; program order puts them
                    # right after the evacuations).
                    hp = t % 2
                    qw = slice(qb * QB, (qb + 1) * QB)
                    for hl in range(2):
                        for g in range(2):
                            nc.sync.dma_start(
                                kq8[hl * 32:(hl + 1) * 32, :, hp, g, qw],
                                zkq[hl * HD + g * 32:hl * HD + g * 32 + 32,
                                    :, hp, qw],
                            )

            def proj_kq(qb, xqb, tiles, evac_act=False):
                for t in tiles:
                    for half in range(2):
                        proj_kq_half(qb, xqb, t, half, evac_act=evac_act)

            def proj_v1(qb, xqb, qt4):
                # ---- projection: v natural for one q-tile of 128 ----
                qt = qb * 4 + qt4
                pvp = proj_ps.tile([P, QB], F32, tag="projps")
                for dt in range(ND):
                    nc.tensor.matmul(
                        pvp[:, :EV],
                        lhsT=(xqb[:, dt, qt4 * P:(qt4 + 1) * P]),
                        rhs=(wv_sb[:, dt, :]),
                        start=(dt == 0), stop=(dt == ND - 1),
                    )
                nc.vector.tensor_tensor(
                    vsb[:, qt, :, 0:HD],
                    pvp[:, :EV].rearrange("p (h d) -> p h d", d=HD),
                    bvb.rearrange("p (h d) -> p h d", d=HD),
                    mybir.AluOpType.add,
                )

            def attn_scores(qb, hp, pt, fillers=(), direct=False):
                # ---- scores + exp for this q block, head pair hp ----
                # Per k-tile: 2 matmuls (even/odd head, PE row-packed at
                # partition bases 0/64) into a 2-bank PSUM tile; one ACT
                # exp evacuates both heads to bf16 pt. The exp chain paces
                # this unit, so `fillers` (next projections + previous
                # unit's PV chains, ~1-3us of PE work each) are drained
                # between k-tiles to keep PE busy during the per-tile exp
                # handoff.
                kt_t = hp
                qt_t = 2 + hp
                kts = list(range(4 * qb, 4 * qb + 4)) + list(range(4 * qb))
                fill = list(fillers)
                nf = 0
                for i, kt in enumerate(kts):
                    r = kt - 4 * qb
                    diag = 0 <= r < 4
                    off = P * r if diag else 0
                    w = QB - off
                    stp = st_ps.tile([P, 2, QB], F32, tag="st")
                    for hl in range(2):
                        if FP8_SCORES and direct and diag:
                            # straight from zkq (2x the PE rows of the
                            # DoubleRow path but no re-layout dependency;
                            # used for the first unit's critical path)
                            base = HD * hl
                            nc.tensor.matmul(
                                stp[:, hl, off:QB],
                                lhsT=zkq[base:base + HD, 0, hp,
                                         kt * P:(kt + 1) * P],
                                rhs=zkq[base:base + HD, 1, hp,
                                        qb * QB + off:(qb + 1) * QB],
                                start=True, stop=True,
                            )
                        elif FP8_SCORES:
                            nc.tensor.matmul(
                                stp[:, hl, off:QB],
                                lhsT=kq8[hl * 32:(hl + 1) * 32, 0, hp, :,
                                         kt * P:(kt + 1) * P],
                                rhs=kq8[hl * 32:(hl + 1) * 32, 1, hp, :,
                                        qb * QB + off:(qb + 1) * QB],
                                start=True, stop=True,
                                perf_mode=mybir.MatmulPerfMode.DoubleRow,
                            )
                        else:
                            base = HD * hl
                            nc.tensor.matmul(
                                stp[:, hl, off:QB],
                                lhsT=zkq[base:base + HD, kt_t,
                                         kt * P:(kt + 1) * P],
                                rhs=zkq[base:base + HD, qt_t,
                                        qb * QB + off:(qb + 1) * QB],
                                start=True, stop=True,
                            )
                    nc.scalar.activation(
                        pt[:, :, kt, off:QB],
                        stp[:, :, off:QB],
                        AF.Exp, scale=SCALE,
                    )
                    if diag:
                        # causal: keep q >= key (iota = jq - p >= 0)
                        nc.gpsimd.affine_select(
                            out=pt[:, 0, kt, off:QB],
                            in_=pt[:, 0, kt, off:QB],
                            compare_op=ALU.is_ge, fill=0.0,
                            base=0, channel_multiplier=-1,
                            pattern=[[1, w]],
                        )
                        nc.vector.tensor_mul(
                            out=pt[:, 1, kt, off:QB],
                            in0=pt[:, 1, kt, off:QB],
                            in1=mask_sb[:, 0:w],
                        )
                    # drain fillers evenly across the remaining k-tiles
                    want = (len(fill) * (i + 1)) // len(kts)
                    while nf < want:
                        fill[nf]()
                        nf += 1
                while nf < len(fill):
                    fill[nf]()
                    nf += 1

            def pv_chunk(qb, hp, qt4, hl, osb, pt, mul_act=False):
                # ---- PV in natural layout: one (q-tile of 128, head) ----
                nkt_q = 4 * qb + qt4 + 1
                # kt order matches the unit's exp completion order
                kts = list(range(4 * qb, nkt_q)) + list(range(0, 4 * qb))
                pvo = pv_ps.tile([P, HD + 1], F32, tag="pv", name="pvo")
                for i, kt in enumerate(kts):
                    nc.tensor.matmul(
                        pvo,
                        lhsT=pt[:, hl, kt, qt4 * P:(qt4 + 1) * P],
                        rhs=vsb[:, kt, 2 * hp + hl, :],
                        start=(i == 0), stop=(i == nkt_q - 1),
                    )
                h = 2 * hp + hl
                rs = work.tile([P, 1], F32, tag="rs")
                nc.vector.reciprocal(rs, pvo[:, HD:HD + 1])
                if mul_act:
                    # ACT-side normalize (exp stream is drained by now)
                    nc.scalar.mul(
                        osb[:, qt4, HD * h:HD * (h + 1)], pvo[:, 0:HD], rs
                    )
                else:
                    nc.vector.tensor_scalar_mul(
                        osb[:, qt4, HD * h:HD * (h + 1)],
                        pvo[:, 0:HD], rs,
                    )

            # ---- global schedule ----
            # Unit order keeps the biggest (most exp-bound) units in the
            # middle where deferred projection work exists to fill PE
            # stalls, and ends on the smallest unit. Each unit's PV
            # chains drain as fillers inside the NEXT unit's score
            # stream (pt is double-buffered), and each unit also carries
            # the projection chunks needed exactly one unit later.
            # Scores of unit (qb, hp) read zkq k-columns of ALL blocks
            # <= qb for that head pair's e-tiles, so every proj_kq chunk
            # must land before its first reader. Projection work is
            # pushed as LATE as precedence allows so the exp stream (the
            # global pacer) starts early and PE has filler work in the
            # exp-bound late units. K-fillers inside a unit that reads
            # them are only safe when they drain before the history
            # k-tiles that need them (even spread puts the first fillers
            # right after the 4 diagonal k-tiles, and relayout latency is
            # covered by the >=4-tile gap to the first reader).
            def K(qb2, tiles, act=False):
                return [
                    (lambda t=t, h=h, q=qb2:
                     proj_kq_half(q, xqbs[q], t, h, evac_act=act))
                    for t in tiles for h in range(2)
                ]

            def V(qb2):
                return [
                    (lambda q4=q4, q=qb2: proj_v1(q, xqbs[q], q4))
                    for q4 in range(4)
                ]

            units = [(0, 0), (1, 0), (2, 0), (3, 0),
                     (3, 1), (2, 1), (1, 1), (0, 1)]
            proj_fill = {
                (0, 0): K(1, (0, 2), act=True),
                (1, 0): K(2, (0, 2)) + V(0),
                (2, 0): K(3, (0, 2)) + K(0, (1, 3)) + V(1),
                (3, 0): K(3, (1, 3)) + K(1, (1, 3)) + V(2),
                (3, 1): K(2, (1, 3)) + V(3),
                (2, 1): [],
                (1, 1): [],
                (0, 1): [],
            }
            # prologue: ONLY block 0's hp0 k/q e-tiles (they gate the
            # first exp); evacuate on ACT which is idle here
            proj_kq(0, xqbs[0], (0, 2), evac_act=True)

            prefetch = {(0, 0): 2, (1, 0): 3}
            osbs = {}
            prev = None
            for ui, (qb, hp) in enumerate(units):
                pt = pts[ui % 2]
                if (qb, hp) in prefetch:
                    nqb = prefetch[(qb, hp)]
                    nc.sync.dma_start(
                        xqbs[nqb], xT_v[:, :, nqb * QB:(nqb + 1) * QB]
                    )
                if hp == 0:
                    osb_t = opool.tile([P, 4, EV], F32, tag="osb", bufs=4,
                                       name=f"osb{qb}")
                    osbs[qb] = osb_t
                fillers = list(proj_fill[(qb, hp)])
                if prev is not None:
                    pqb, php, ppt = prev
                    fillers += [
                        lambda q4=q4, hl=hl, a=pqb, b=php, c=ppt:
                            pv_chunk(a, b, q4, hl, osbs[a], c)
                        for q4 in range(4) for hl in range(2)
                    ]
                if ui == len(units) - 1:
                    # last unit: its own PV chains interleave with its own
                    # exps. Own chunk for q-tile qt only needs exps up to
                    # kt=qt, so lead each group of 4 with it (the spread
                    # drains 4 fillers per k-tile here); normalize muls
                    # alternate DVE/gpsimd to halve the closing cascade.
                    fillers += [
                        lambda q4=q4, hl=hl:
                            pv_chunk(qb, hp, q4, hl, osbs[qb], pt,
                                     mul_act=False)
                        for q4 in range(4) for hl in range(2)
                    ]
                attn_scores(qb, hp, pt, fillers,
                            direct=(hp == 0 and qb <= 1))
                if prev is not None:
                    # half of osb (one head pair) is complete once the
                    # previous unit's PV chains have drained
                    nc.sync.dma_start(
                        o_v[:, pqb * 4:(pqb + 1) * 4, php * P:(php + 1) * P],
                        osbs[pqb][:, :, php * P:(php + 1) * P],
                    )
                prev = (qb, hp, pt)
            nc.sync.dma_start(o_v[:, 0:4, P:2 * P], osbs[0][:, :, P:2 * P])
    if split_waits:
        _split_matmul_waits(nc)
    return nc


_nc_cache = None


def _get_nc():
    global _nc_cache
    if _nc_cache is None:
        _nc_cache = build_nc()
    return _nc_cache


def make_in_maps(x, W, b):
    import ml_dtypes

    x = np.asarray(x, dtype=np.float32)
    W = np.asarray(W, dtype=np.float32)
    b = np.asarray(b, dtype=np.float32)
    bf = ml_dtypes.bfloat16
    in_maps = []
    xTs = [np.ascontiguousarray(x[n].T.astype(bf)) for n in range(N)]
    for c in range(8):
        n, g = divmod(c, 4)
        rk = slice(256 * g, 256 * g + 256)
        rq = slice(D + 256 * g, D + 256 * g + 256)
        rv = slice(2 * D + 256 * g, 2 * D + 256 * g + 256)
        wkq = np.ascontiguousarray(
            np.concatenate([W[rk], W[rq]], axis=0).T.astype(bf)
        )
        wv = np.ascontiguousarray(W[rv].T.astype(bf))
        bkq = np.ascontiguousarray(
            np.concatenate([b[rk], b[rq]]).reshape(4, P).T
        )
        bv = np.ascontiguousarray(b[rv].reshape(1, EV))
        in_maps.append(
            {"xT": xTs[n], "wkq": wkq, "wv": wv, "bkq": bkq, "bv": bv}
        )
    return in_maps


def run(inputs, **kwargs):
    nc = _get_nc()
    in_maps = make_in_maps(inputs["x"], inputs["W"], inputs["b"])
    res = run_bass_kernel_spmd(nc, in_maps, core_ids=list(range(8)), **kwargs)
    out = np.empty((N, S, D), dtype=np.float32)
    for c in range(8):
        n, g = divmod(c, 4)
        out[n, :, 256 * g:256 * g + 256] = res.results[c]["o"]
    return out, res


def kernel(**inputs):
    out, _ = run(inputs)
    return out
